# revision 1
# baseline (speedup 1.0000x reference)
"""ConvAttention TRN2 kernel: depthwise-conv QKV + full softmax attention + projection.

Self-contained: hardcodes shapes B=2, C=96, H=W=64, N=4096, heads=3, d=32.
Shards query rows across 8 NeuronCores (512 rows each); k/v conv replicated
on-device; q conv computed from a per-core halo slice of the input.
"""

import os
import sys

import numpy as np

for _p in ("/opt/trn_rl_repo", "/root/.axon_site/_ro/trn_rl_repo"):
    if os.path.isdir(_p) and _p not in sys.path:
        sys.path.append(_p)

from contextlib import ExitStack

import concourse.bass as bass
import concourse.masks as masks
import concourse.tile as tile
from concourse import bacc, mybir
from concourse.bass_utils import run_bass_kernel_spmd

F32 = mybir.dt.float32
F32R = mybir.dt.float32r
BF16 = mybir.dt.bfloat16

B = 2
C = 96
H = W = 64
N = H * W            # 4096
NHEADS = 3
D = C // NHEADS      # 32
SCALE = float(D) ** -0.5
NCORES = 8
NQ = N // NCORES     # 512 query rows per core
QROWS = NQ // W      # 8 spatial rows per core
WP = W + 2           # padded width 66
NKCH = N // 128      # 32 key chunks of 128


def _build_program(debug_outputs=False):
    nc = bacc.Bacc("TRN2", target_bir_lowering=False, debug=False, num_devices=NCORES)

    xq_d = nc.dram_tensor("xq", [B, 32, QROWS + 4, WP], BF16, kind="ExternalInput").ap()
    xkv_d = nc.dram_tensor("xkv", [B, 64, H + 4, WP], BF16, kind="ExternalInput").ap()
    wm_d = nc.dram_tensor("wm", [3, 3, 97, 96], BF16, kind="ExternalInput").ap()
    pw_d = nc.dram_tensor("pw", [96, 96], BF16, kind="ExternalInput").ap()
    pb_d = nc.dram_tensor("pb", [96, 1], F32, kind="ExternalInput").ap()
    y_d = nc.dram_tensor("y", [B, N, 96], F32, kind="ExternalOutput").ap()
    # per-(b,h) collective staging: ah block [32, 512] -> allgather -> flat scrambled layout
    stg_d = [nc.dram_tensor(f"stg{b}", [96, NQ], F32).ap() for b in range(B)]
    gth_d = [[nc.dram_tensor(f"gth{b}_{h}", [NCORES, 32, NQ], F32,
                             addr_space="Shared").ap()
              for h in range(NHEADS)] for b in range(B)]
    flt_d = [nc.dram_tensor(f"flt{b}", [96, NCORES, NQ], F32).ap() for b in range(B)]
    dbg = {}
    if debug_outputs:
        dbg["q"] = nc.dram_tensor("dbg_q", [96, B, NQ], BF16, kind="ExternalOutput").ap()
        dbg["k"] = nc.dram_tensor("dbg_k", [96, B, N], BF16, kind="ExternalOutput").ap()
        dbg["vt"] = nc.dram_tensor("dbg_vt", [128, B, NHEADS, NKCH, 33], BF16, kind="ExternalOutput").ap()
        dbg["acc"] = nc.dram_tensor("dbg_acc", [B, NHEADS, 33, 512], F32, kind="ExternalOutput").ap()
        dbg["ah"] = nc.dram_tensor("dbg_ah", [B, NHEADS, 32, 512], F32, kind="ExternalOutput").ap()

    LQ = QROWS * WP          # 528 usable elems per (dy) shift for q
    LK = (H + 2) * WP        # 4356 for k/v

    with tile.TileContext(nc) as tc, ExitStack() as ctx:
        consts = ctx.enter_context(tc.tile_pool(name="consts", bufs=1))
        xrep_p = ctx.enter_context(tc.tile_pool(name="xrep", bufs=1))
        qkv_p = ctx.enter_context(tc.tile_pool(name="qkv", bufs=1))
        vtmp_p = ctx.enter_context(tc.tile_pool(name="vtmp", bufs=2))
        vt_p = ctx.enter_context(tc.tile_pool(name="vt", bufs=1))
        exp_p = ctx.enter_context(tc.tile_pool(name="exp", bufs=18))
        arhs_p = ctx.enter_context(tc.tile_pool(name="arhs", bufs=2))
        small_p = ctx.enter_context(tc.tile_pool(name="small", bufs=2))
        out_p = ctx.enter_context(tc.tile_pool(name="out", bufs=2))

        conv_ps = ctx.enter_context(tc.tile_pool(name="conv_ps", bufs=2, space="PSUM"))
        acc_ps_p = conv_ps
        sc_ps_p = ctx.enter_context(tc.tile_pool(name="sc_ps", bufs=2, space="PSUM"))
        misc_ps_p = ctx.enter_context(tc.tile_pool(name="misc_ps", bufs=2, space="PSUM"))

        # ---- constants ----
        wm_sb = consts.tile([97, 9, 96], BF16)
        for g in range(3):
            for dx in range(3):
                nc.sync.dma_start(wm_sb[:, g * 3 + dx, :], wm_d[g, dx, :, :])
        pw_sb = consts.tile([96, 96], BF16)
        nc.sync.dma_start(pw_sb[:], pw_d[:, :])
        pb_sb = consts.tile([96, 1], F32)
        nc.sync.dma_start(pb_sb[:], pb_d[:, :])
        ident = consts.tile([128, 128], BF16)
        masks.make_identity(nc, ident[:])
        identf = consts.tile([128, 128], F32)
        masks.make_identity(nc, identf[:])
        ones_col = consts.tile([1, 32], F32)
        nc.vector.memset(ones_col[:], 1.0)

        # ---- x replicated-shift loads: partition p = dy*32 + c holds channel c shifted dy rows ----
        xrep_q = xrep_p.tile([97, B, LQ], BF16)
        xrep_k = xrep_p.tile([97, B, LK], BF16)
        xrep_v = xrep_p.tile([97, B, LK], BF16)
        xq_flat = xq_d[:, :, :, :].rearrange("b c r w -> c b (r w)")
        xkv_flat = xkv_d[:, :, :, :].rearrange("b c r w -> c b (r w)")
        for dy in range(3):
            nc.sync.dma_start(
                xrep_q[dy * 32:(dy + 1) * 32, :, :],
                xq_flat[:, :, dy * WP: dy * WP + LQ])
            for b in range(B):
                nc.sync.dma_start(
                    xrep_k[dy * 32:(dy + 1) * 32, b, :],
                    xkv_flat[0:32, b, dy * WP: dy * WP + LK])
                nc.gpsimd.dma_start(
                    xrep_v[dy * 32:(dy + 1) * 32, b, :],
                    xkv_flat[32:64, b, dy * WP: dy * WP + LK])
        nc.vector.memset(xrep_q[96:97, :, :], 1.0)
        nc.vector.memset(xrep_k[96:97, :, :], 1.0)
        nc.vector.memset(xrep_v[96:97, :, :], 1.0)

        # ---- qkv persistent sbuf ----
        q_all = qkv_p.tile([96, B, NQ], BF16)     # (h*32+d, b, nq)
        k_all = qkv_p.tile([96, B, N], BF16)      # (h*32+d, b, nk)
        vt_aug = vt_p.tile([128, B, NHEADS, NKCH, 33], BF16)  # (nk%128, b, h, chunk, d|1)
        nc.vector.memset(vt_aug[:, :, :, :, 32:33], 1.0)

        # ---- depthwise conv as matmuls: out[o, n] = sum_{dy,c} W[dy*32+c, o] * xrep[(dy,c), n+dx-shift] ----
        def conv_group(g, xr, nblocks, rows_per_blk, emit_block):
            view = [None] * B
            for b in range(B):
                view[b] = xr[:, b, :].rearrange("k (r w) -> k r w", w=WP)
            for b in range(B):
                for blk in range(nblocks):
                    ps = conv_ps.tile([96, 512], F32, tag="pacc")
                    for dx in range(3):
                        rhs = view[b][:, blk * rows_per_blk: blk * rows_per_blk + rows_per_blk, dx: dx + W]
                        nc.tensor.matmul(
                            ps[:, :],
                            lhsT=wm_sb[:, g * 3 + dx, :],
                            rhs=rhs,
                            start=(dx == 0), stop=(dx == 2))
                    emit_block(b, blk, ps)

        # q: one 512-block per b
        conv_group(0, xrep_q, 1, QROWS,
                   lambda b, blk, ps: nc.scalar.copy(q_all[:, b, :], ps[:, :]))
        # k: 8 blocks per b
        conv_group(1, xrep_k, 8, QROWS,
                   lambda b, blk, ps: nc.scalar.copy(k_all[:, b, blk * 512:(blk + 1) * 512], ps[:, :]))

        # v: 8 blocks per b -> transpose into vt_aug
        def emit_v(b, blk, ps):
            vtmp = vtmp_p.tile([96, 512], BF16)
            nc.scalar.copy(vtmp[:, :], ps[:, :])
            for c4 in range(4):
                ch = blk * 4 + c4
                tp = misc_ps_p.tile([128, 96], BF16, tag="m")
                nc.tensor.transpose(tp[:, :], vtmp[:, c4 * 128:(c4 + 1) * 128], ident[0:96, 0:96])
                nc.vector.tensor_copy(
                    vt_aug[:, b, :, ch, 0:32],
                    tp[:, :].rearrange("p (h d) -> p h d", d=32))
        conv_group(2, xrep_v, 8, QROWS, emit_v)

        # ---- attention + gather (both batches) ----
        for b in range(B):
            ah_tiles = []
            for h in range(NHEADS):
                # stage 1: stream all score matmuls (pairs share a 2-bank psum
                # tile so each exp covers 1024 columns)
                exs = []
                for cp in range(NKCH // 2):
                    sc2 = sc_ps_p.tile([128, 1024], F32)
                    for half in range(2):
                        ch = 2 * cp + half
                        nc.tensor.matmul(
                            sc2[:, half * 512:(half + 1) * 512],
                            lhsT=k_all[32 * h:32 * (h + 1), b, ch * 128:(ch + 1) * 128],
                            rhs=q_all[32 * h:32 * (h + 1), b, :],
                            start=True, stop=True)
                    ex2 = exp_p.tile([128, 1024], BF16)
                    nc.scalar.activation(ex2[:, :], sc2[:, :],
                                         mybir.ActivationFunctionType.Exp, scale=SCALE)
                    exs.append(ex2)
                # stage 2: accumulate PV back-to-back
                acc = acc_ps_p.tile([33, 512], F32, tag="pacc")
                for cp in range(NKCH // 2):
                    for half in range(2):
                        ch = 2 * cp + half
                        nc.tensor.matmul(
                            acc[:, :],
                            lhsT=vt_aug[:, b, h, ch, :],
                            rhs=exs[cp][:, half * 512:(half + 1) * 512],
                            start=(ch == 0), stop=(ch == NKCH - 1))
                # normalize: rows 0:32 are sum(exp * v), row 32 is sum(exp)
                rden = small_p.tile([1, 512], F32)
                nc.vector.reciprocal(rden[:, :], acc[32:33, :])
                bc = misc_ps_p.tile([32, 512], F32, tag="m")
                nc.tensor.matmul(bc[:, :], lhsT=ones_col[:, :],
                                 rhs=rden[:, :], start=True, stop=True)
                num = small_p.tile([32, 512], F32, tag="num")
                nc.vector.tensor_copy(num[:, :], acc[0:32, :])
                ah = arhs_p.tile([32, 512], F32, tag=f"a{h}")
                nc.vector.tensor_mul(ah[:, :], num[:, :], bc[:, :])
                ah_tiles.append(ah)
                nc.gpsimd.dma_start(stg_d[b][32 * h:32 * (h + 1), :], ah[:, :])
                if debug_outputs:
                    accs = small_p.tile([33, 512], F32, tag="dbgacc")
                    nc.vector.tensor_copy(accs[:, :], acc[:, :])
                    nc.sync.dma_start(dbg["acc"][b, h, :, :], accs[:, :])
                    nc.sync.dma_start(dbg["ah"][b, h, :, :], ah[:, :])
                # gather this head's token blocks from all cores, rebuild the
                # channel-major flat layout (reference reshape(B,N,C) flattens
                # (h,d,N) row-major)
                nc.gpsimd.collective_compute(
                    "AllGather", mybir.AluOpType.bypass,
                    ins=[stg_d[b][32 * h:32 * (h + 1), :]],
                    outs=[gth_d[b][h][:, :, :]],
                    replica_groups=[list(range(NCORES))])
                nc.sync.dma_start(
                    flt_d[b][32 * h:32 * (h + 1), :, :],
                    gth_d[b][h][:, :, :].rearrange("j c n -> c j n"))

        # ---- projection (both batches), after all collectives are in flight ----
        for b in range(B):
            # scrambled projection input: out2[n', c'] = flat[n' * 96 + c']
            out2 = flt_d[b][:, :, :].rearrange("c j n -> (c j n)").rearrange(
                "(n c) -> n c", c=96)
            yin_all = out_p.tile([128, NKCH, 96], F32, tag="yin")
            nc.sync.dma_start(yin_all[:, :, :],
                              out2.rearrange("(ch p) c -> p ch c", p=128))
            yo_all = out_p.tile([128, NKCH, 96], F32, tag="yo")
            for cg in range(NCORES):
                rhs = small_p.tile([96, 512], BF16, tag="prhs")
                for c4 in range(4):
                    chk = cg * 4 + c4
                    tpi = misc_ps_p.tile([96, 128], F32, tag="m")
                    nc.tensor.transpose(tpi[:, :], yin_all[:, chk, :], identf[:, :])
                    nc.vector.tensor_copy(rhs[:, c4 * 128:(c4 + 1) * 128], tpi[:, :])
                yps = misc_ps_p.tile([96, 512], F32, tag="m")
                nc.tensor.matmul(yps[:, :], lhsT=pw_sb[:, :], rhs=rhs[:, :],
                                 start=True, stop=True)
                ysb = small_p.tile([96, 512], BF16, tag="ysb")
                nc.vector.tensor_scalar_add(ysb[:, :], yps[:, :], pb_sb[:, :])
                for c4 in range(4):
                    chk = cg * 4 + c4
                    tp = misc_ps_p.tile([128, 96], BF16, tag="m")
                    nc.tensor.transpose(tp[:, :], ysb[:, c4 * 128:(c4 + 1) * 128], ident[0:96, 0:96])
                    nc.vector.tensor_copy(yo_all[:, chk, :], tp[:, :])
            nc.sync.dma_start(
                y_d[b].rearrange("(ch p) c -> p ch c", p=128), yo_all[:, :, :])

        if debug_outputs:
            nc.sync.dma_start(dbg["q"][:, :, :], q_all[:, :, :])
            nc.sync.dma_start(dbg["k"][:, :, :], k_all[:, :, :])
            nc.sync.dma_start(dbg["vt"][:, :, :, :, :], vt_aug[:, :, :, :, :])

    nc.compile()
    return nc


_PROG = None


def _prep_inputs(x, qkv_w, qkv_b, proj_w, proj_b):
    x = np.asarray(x, np.float32)
    qkv_w = np.asarray(qkv_w, np.float32)
    qkv_b = np.asarray(qkv_b, np.float32)
    proj_w = np.asarray(proj_w, np.float32)
    proj_b = np.asarray(proj_b, np.float32)

    xt = x.transpose(0, 2, 1).reshape(B, C, H, W)
    xpad = np.zeros((B, C, H + 2, WP), np.float32)
    xpad[:, :, 1:H + 1, 1:W + 1] = xt

    xkv = np.zeros((B, 64, H + 4, WP), np.float32)
    xkv[:, :, 0:H + 2, :] = xpad[:, 32:96]

    xqs = []
    for i in range(NCORES):
        buf = np.zeros((B, 32, QROWS + 4, WP), np.float32)
        buf[:, :, 0:QROWS + 2, :] = xpad[:, 0:32, i * QROWS: i * QROWS + QROWS + 2, :]
        xqs.append(buf)

    w = qkv_w.reshape(3 * C, 3, 3)
    wm = np.zeros((3, 3, 97, 96), np.float32)  # [g, dx, k=(dy*32+c), o]
    o = np.arange(96)
    for g in range(3):
        for dy in range(3):
            for dx in range(3):
                wm[g, dx, dy * 32 + o // 3, o] = w[g * 96 + o, dy, dx]
        wm[g, 0, 96, :] = qkv_b[g * 96:(g + 1) * 96]

    import ml_dtypes
    bf16 = ml_dtypes.bfloat16
    xqs = [a.astype(bf16) for a in xqs]
    xkv = xkv.astype(bf16)
    wm = wm.astype(bf16)
    pw = np.ascontiguousarray(proj_w.T).astype(bf16)
    pb = np.ascontiguousarray(proj_b.reshape(96, 1))
    return xqs, xkv, wm, pw, pb


def kernel(x, qkv_w, qkv_b, proj_w, proj_b, H=64, W=64):
    global _PROG
    if _PROG is None:
        _PROG = _build_program()
    nc = _PROG

    xqs, xkv, wm, pw, pb = _prep_inputs(x, qkv_w, qkv_b, proj_w, proj_b)
    in_maps = [
        {"xq": xqs[i], "xkv": xkv, "wm": wm, "pw": pw, "pb": pb}
        for i in range(NCORES)
    ]
    res = run_bass_kernel_spmd(nc, in_maps, list(range(NCORES)))
    return np.asarray(res.results[0]["y"])



# revision 16
# speedup vs baseline: 1.0213x; 1.0213x over previous
"""ConvAttention TRN2 kernel: depthwise-conv QKV + full softmax attention + projection.

Self-contained: hardcodes shapes B=2, C=96, H=W=64, N=4096, heads=3, d=32.
v2 design:
  - Each core computes q/k/v conv ONLY for its own 512-token block (8 spatial
    rows + halo); k and v^T blocks are AllGathered (bf16) so every core holds
    full k / v for its 512 query rows.
  - Attention: per (b,h): 32 QK chunk matmuls -> exp (scalar engine) -> PV
    accumulation with augmented ones-row for the softmax denominator.
  - Normalization is issued deferred (reciprocal on vector, broadcast via a
    tiny matmul placed behind the next head's QK stream) so the in-order
    tensor queue never stalls on it.
  - Projection is local per core (per-head accumulating matmuls); the full
    output is assembled host-side from the 8 cores' y slices.
"""

import os
import sys

import numpy as np

for _p in ("/opt/trn_rl_repo", "/root/.axon_site/_ro/trn_rl_repo"):
    if os.path.isdir(_p) and _p not in sys.path:
        sys.path.append(_p)

from contextlib import ExitStack

import concourse.bass as bass
import concourse.masks as masks
import concourse.tile as tile
from concourse import bacc, mybir
from concourse.bass_utils import run_bass_kernel_spmd

F32 = mybir.dt.float32
BF16 = mybir.dt.bfloat16

B = 2
C = 96
H = W = 64
N = H * W            # 4096
NHEADS = 3
D = C // NHEADS      # 32
SCALE = float(D) ** -0.5
NCORES = 8
NQ = N // NCORES     # 512 query rows per core
QROWS = NQ // W      # 8 spatial rows per core
WP = W + 2           # padded width 66
NKCH = N // 128      # 32 key chunks of 128
HROWS = QROWS + 2    # halo rows per core
LH = QROWS * WP      # 528 usable elems per dy shift


def _build_program(debug_outputs=False):
    nc = bacc.Bacc("TRN2", target_bir_lowering=False, debug=False, num_devices=NCORES)

    xh_d = nc.dram_tensor("xh", [B, 96, HROWS, WP], BF16, kind="ExternalInput").ap()
    wm_d = nc.dram_tensor("wm", [3, 3, 97, 96], BF16, kind="ExternalInput").ap()
    pw_d = nc.dram_tensor("pw", [96, 96], BF16, kind="ExternalInput").ap()
    pb_d = nc.dram_tensor("pb", [96, 1], F32, kind="ExternalInput").ap()
    ridx_d = nc.dram_tensor("ridx", [128, 1], mybir.dt.int32,
                            kind="ExternalInput").ap()
    y_d = nc.dram_tensor("y", [B, NQ, 96], F32, kind="ExternalOutput").ap()

    kstg_d = [nc.dram_tensor(f"kstg{b}", [96, NQ], BF16).ap() for b in range(B)]
    vstg_d = [nc.dram_tensor(f"vstg{b}", [128, NHEADS, 4, 33], BF16).ap()
              for b in range(B)]
    astg_d = [nc.dram_tensor(f"astg{b}", [96, NQ], BF16).ap() for b in range(B)]
    kg_d = [nc.dram_tensor(f"kg{b}", [NCORES, 96, NQ], BF16,
                           addr_space="Shared").ap() for b in range(B)]
    vg_d = [nc.dram_tensor(f"vg{b}", [NCORES, 128, NHEADS, 4, 33], BF16,
                           addr_space="Shared").ap() for b in range(B)]
    ag_d = [nc.dram_tensor(f"ag{b}", [NCORES, 96, NQ], BF16,
                           addr_space="Shared").ap() for b in range(B)]
    flat_d = [nc.dram_tensor(f"flat{b}", [96, N], BF16).ap() for b in range(B)]
    dbg = {}
    if debug_outputs:
        dbg["q"] = nc.dram_tensor("dbg_q", [96, B, NQ], BF16, kind="ExternalOutput").ap()
        dbg["k"] = nc.dram_tensor("dbg_k", [96, B, N], BF16, kind="ExternalOutput").ap()
        dbg["vt"] = nc.dram_tensor("dbg_vt", [128, B, NHEADS, NKCH, 33], BF16, kind="ExternalOutput").ap()
        dbg["ah"] = nc.dram_tensor("dbg_ah", [B, NHEADS, 32, 512], F32, kind="ExternalOutput").ap()

    with tile.TileContext(nc) as tc, ExitStack() as ctx:
        consts = ctx.enter_context(tc.tile_pool(name="consts", bufs=1))
        xrep_p = ctx.enter_context(tc.tile_pool(name="xrep", bufs=1))
        qkv_p = ctx.enter_context(tc.tile_pool(name="qkv", bufs=1))
        vtmp_p = ctx.enter_context(tc.tile_pool(name="vtmp", bufs=2))
        exp_p = ctx.enter_context(tc.tile_pool(name="exp", bufs=18))
        small_p = ctx.enter_context(tc.tile_pool(name="small", bufs=3))
        ah_p = ctx.enter_context(tc.tile_pool(name="ah", bufs=6))
        out_p = ctx.enter_context(tc.tile_pool(name="out", bufs=2))

        acc_ps = ctx.enter_context(tc.tile_pool(name="acc_ps", bufs=2, space="PSUM"))
        sc_ps = ctx.enter_context(tc.tile_pool(name="sc_ps", bufs=2, space="PSUM"))
        misc_ps = ctx.enter_context(tc.tile_pool(name="misc_ps", bufs=2, space="PSUM"))

        # ---- constants ----
        wm_sb = consts.tile([97, 9, 96], BF16)
        for g in range(3):
            for dx in range(3):
                nc.sync.dma_start(wm_sb[:, g * 3 + dx, :], wm_d[g, dx, :, :])
        pw_sb = consts.tile([96, 96], BF16)
        nc.sync.dma_start(pw_sb[:, :], pw_d[:, :])
        pb_sb = consts.tile([96, 1], F32)
        nc.sync.dma_start(pb_sb[:], pb_d[:, :])
        ridx_sb = consts.tile([128, 1], mybir.dt.int32)
        nc.sync.dma_start(ridx_sb[:, :], ridx_d[:, :])
        ident = consts.tile([128, 128], BF16)
        masks.make_identity(nc, ident[:])
        ones_col = consts.tile([1, 32], F32)
        nc.vector.memset(ones_col[:], 1.0)

        # ---- halo input, replicated-shift layout: partition dy*32+c = channel c shifted dy rows ----
        xr = {}
        for gname, g in (("q", 0), ("k", 1), ("v", 2)):
            t = xrep_p.tile([97, B, LH], BF16, tag=f"x{gname}")
            xr[gname] = t
            flat = xh_d[:, g * 32:(g + 1) * 32, :, :].rearrange("b c r w -> c b (r w)")
            for dy in range(3):
                nc.sync.dma_start(t[dy * 32:(dy + 1) * 32, :, :],
                                  flat[:, :, dy * WP: dy * WP + LH])
            nc.vector.memset(t[96:97, :, :], 1.0)

        # ---- conv for one group/batch: psum [96, 512] ----
        def conv(g, xrt, b):
            view = xrt[:, b, :].rearrange("k (r w) -> k r w", w=WP)
            ps = acc_ps.tile([96, 512], F32, tag="pacc")
            for dx in range(3):
                nc.tensor.matmul(ps[:, :], lhsT=wm_sb[:, g * 3 + dx, :],
                                 rhs=view[:, 0:QROWS, dx: dx + W],
                                 start=(dx == 0), stop=(dx == 2))
            return ps

        # ---- k conv -> stage -> gather (per b) ----
        kblk = qkv_p.tile([96, B, 512], BF16, tag="kblk")
        for b in range(B):
            ps = conv(1, xr["k"], b)
            nc.vector.tensor_copy(kblk[:, b, :], ps[:, :])
            nc.sync.dma_start(kstg_d[b][:, :], kblk[:, b, :])
            nc.gpsimd.collective_compute(
                "AllGather", mybir.AluOpType.bypass,
                ins=[kstg_d[b][:, :]],
                outs=[kg_d[b][:, :, :]],
                replica_groups=[list(range(NCORES))])

        # ---- v conv -> transpose -> stage -> gather (per b) ----
        vstg_sb = qkv_p.tile([128, B, NHEADS, 4, 33], BF16, tag="vstg")
        nc.vector.memset(vstg_sb[:, :, :, :, 32:33], 1.0)
        for b in range(B):
            ps = conv(2, xr["v"], b)
            vtmp = vtmp_p.tile([96, 512], BF16)
            nc.scalar.copy(vtmp[:, :], ps[:, :])
            for c4 in range(4):
                tp = misc_ps.tile([128, 96], BF16, tag="m")
                nc.tensor.transpose(tp[:, :], vtmp[:, c4 * 128:(c4 + 1) * 128],
                                    ident[0:96, 0:96])
                nc.vector.tensor_copy(
                    vstg_sb[:, b, :, c4, 0:32],
                    tp[:, :].rearrange("p (h d) -> p h d", d=32))
            nc.sync.dma_start(vstg_d[b][:, :, :, :], vstg_sb[:, b, :, :, :])
            nc.gpsimd.collective_compute(
                "AllGather", mybir.AluOpType.bypass,
                ins=[vstg_d[b][:, :, :, :]],
                outs=[vg_d[b][:, :, :, :, :]],
                replica_groups=[list(range(NCORES))])

        # ---- q conv ----
        q_small = qkv_p.tile([96, B, 512], BF16, tag="qsm")
        for b in range(B):
            ps = conv(0, xr["q"], b)
            nc.vector.tensor_copy(q_small[:, b, :], ps[:, :])

        # ---- gather reads (on scalar queue; attention waits on these anyway) ----
        k_all = qkv_p.tile([96, B, N], BF16, tag="kall")
        vt_aug = qkv_p.tile([128, B, NHEADS, NKCH, 33], BF16, tag="vt")
        for b in range(B):
            nc.scalar.dma_start(
                k_all[:, b, :].rearrange("c (j n) -> c j n", j=NCORES),
                kg_d[b][:, :, :].rearrange("j c n -> c j n"))
            nc.scalar.dma_start(
                vt_aug[:, b, :, :, :].rearrange("p h (j c) s -> p h j c s", j=NCORES),
                vg_d[b][:, :, :, :, :].rearrange("j p h c s -> p h j c s"))

        # ---- attention ----
        BH = [(b, h) for b in range(B) for h in range(NHEADS)]
        pending = [None]   # (acc, rden, b, h)

        def flush_norm():
            if pending[0] is None:
                return
            acc, rden, b, h = pending[0]
            pending[0] = None
            bc = misc_ps.tile([32, 512], F32, tag="m")
            nc.tensor.matmul(bc[:, :], lhsT=ones_col[:, :], rhs=rden[:, :],
                             start=True, stop=True)
            bc_sb = small_p.tile([32, 512], F32, tag="bcsb")
            nc.vector.tensor_copy(bc_sb[:, :], bc[:, :])
            ah = ah_p.tile([32, 512], BF16, tag=f"a{b}_{h}")
            nc.vector.tensor_mul(ah[:, :], acc[0:32, :], bc_sb[:, :])
            nc.sync.dma_start(astg_d[b][32 * h:32 * (h + 1), :], ah[:, :])
            if debug_outputs:
                ahf = small_p.tile([32, 512], F32, tag="dbgah")
                nc.vector.tensor_copy(ahf[:, :], ah[:, :])
                nc.sync.dma_start(dbg["ah"][b, h, :, :], ahf[:, :])
            if h == NHEADS - 1:
                emit_proj(b)

        def emit_proj(b):
            # reference reshape(B, N, C) flattens (h, d, n) row-major: output
            # row n' = flat[96n' : 96n'+96]. This core projects rows
            # [512j, 512j+512): one indirect gather of 128 rows x 384 elems
            # (4 windows each) using the per-core row-index input.
            nc.gpsimd.collective_compute(
                "AllGather", mybir.AluOpType.bypass,
                ins=[astg_d[b][:, :]],
                outs=[ag_d[b][:, :, :]],
                replica_groups=[list(range(NCORES))])
            nc.gpsimd.dma_start(
                flat_d[b][:, :].rearrange("c (i n) -> c i n", i=NCORES),
                ag_d[b][:, :, :].rearrange("i c n -> c i n"))
            win = out_p.tile([128, 4, 96], BF16, tag="win")
            nc.gpsimd.indirect_dma_start(
                out=win[:, :, :].rearrange("p g c -> p (g c)"),
                out_offset=None,
                in_=flat_d[b][:, :].rearrange("c n -> (c n)").rearrange(
                    "(r e) -> r e", e=384),
                in_offset=bass.IndirectOffsetOnAxis(ap=ridx_sb[:, :1], axis=0))
            rhs = out_p.tile([96, 512], BF16, tag="prhs")
            for g in range(4):
                tpi = misc_ps.tile([96, 128], BF16, tag="m")
                nc.tensor.transpose(tpi[:, :], win[:, g, :], ident[:, :])
                nc.vector.tensor_copy(rhs[:, g * 128:(g + 1) * 128], tpi[:, :])
            y_ps = misc_ps.tile([96, 512], F32, tag="m")
            nc.tensor.matmul(y_ps[:, :], lhsT=pw_sb[:, :], rhs=rhs[:, :],
                             start=True, stop=True)
            ysb = out_p.tile([96, 512], BF16, tag="ysb")
            nc.vector.tensor_scalar_add(ysb[:, :], y_ps[:, :], pb_sb[:, :])
            yo = out_p.tile([128, 4, 96], F32, tag="yo")
            for g in range(4):
                tp = misc_ps.tile([128, 96], BF16, tag="m")
                nc.tensor.transpose(tp[:, :], ysb[:, g * 128:(g + 1) * 128],
                                    ident[0:96, 0:96])
                nc.vector.tensor_copy(yo[:, g, :], tp[:, :])
            nc.sync.dma_start(
                y_d[b].rearrange("(p g) c -> p g c", g=4), yo[:, :, :])

        for b, h in BH:
            # stage 1: QK chunk matmuls, paired into 2-bank psum tiles + exp
            exs = []
            for cp in range(NKCH // 2):
                sc2 = sc_ps.tile([128, 1024], F32)
                for half in range(2):
                    ch = 2 * cp + half
                    nc.tensor.matmul(
                        sc2[:, half * 512:(half + 1) * 512],
                        lhsT=k_all[32 * h:32 * (h + 1), b, ch * 128:(ch + 1) * 128],
                        rhs=q_small[32 * h:32 * (h + 1), b, :],
                        start=True, stop=True)
                ex2 = exp_p.tile([128, 1024], BF16)
                nc.scalar.activation(ex2[:, :], sc2[:, :],
                                     mybir.ActivationFunctionType.Exp, scale=SCALE)
                exs.append(ex2)
                if cp == 3:
                    # previous head's normalization: bc matmul lands behind 8
                    # queued QK matmuls, after its reciprocal has finished
                    flush_norm()
            # stage 2: PV accumulation (row 32 = sum of exp)
            acc = acc_ps.tile([33, 512], F32, tag="pacc")
            for cp in range(NKCH // 2):
                for half in range(2):
                    ch = 2 * cp + half
                    nc.tensor.matmul(
                        acc[:, :],
                        lhsT=vt_aug[:, b, h, ch, :],
                        rhs=exs[cp][:, half * 512:(half + 1) * 512],
                        start=(ch == 0), stop=(ch == NKCH - 1))
            den_sb = small_p.tile([1, 512], F32, tag="densb")
            nc.vector.tensor_copy(den_sb[:, :], acc[32:33, :])
            rden = small_p.tile([1, 512], F32, tag="rden")
            nc.vector.reciprocal_approx_fast(rden[:, :], den_sb[:, :])
            pending[0] = (acc, rden, b, h)
        flush_norm()

        if debug_outputs:
            nc.sync.dma_start(dbg["q"][:, :, :], q_small[:, :, :])
            nc.sync.dma_start(dbg["k"][:, :, :], k_all[:, :, :])
            nc.sync.dma_start(dbg["vt"][:, :, :, :, :], vt_aug[:, :, :, :, :])

    nc.compile()
    return nc


_PROG = None


def _prep_inputs(x, qkv_w, qkv_b, proj_w, proj_b):
    import ml_dtypes
    bf16 = ml_dtypes.bfloat16

    x = np.asarray(x, np.float32)
    qkv_w = np.asarray(qkv_w, np.float32)
    qkv_b = np.asarray(qkv_b, np.float32)
    proj_w = np.asarray(proj_w, np.float32)
    proj_b = np.asarray(proj_b, np.float32)

    xt = x.transpose(0, 2, 1).reshape(B, C, H, W)
    xpad = np.zeros((B, C, H + 2, WP), np.float32)
    xpad[:, :, 1:H + 1, 1:W + 1] = xt
    xpad = xpad.astype(bf16)

    xhs = [np.ascontiguousarray(xpad[:, :, i * QROWS: i * QROWS + HROWS, :])
           for i in range(NCORES)]

    w = qkv_w.reshape(3 * C, 3, 3)
    wm = np.zeros((3, 3, 97, 96), np.float32)  # [g, dx, k=(dy*32+c), o]
    o = np.arange(96)
    for g in range(3):
        for dy in range(3):
            for dx in range(3):
                wm[g, dx, dy * 32 + o // 3, o] = w[g * 96 + o, dy, dx]
        wm[g, 0, 96, :] = qkv_b[g * 96:(g + 1) * 96]
    wm = wm.astype(bf16)

    pw = np.ascontiguousarray(proj_w.T).astype(bf16)
    pb = np.ascontiguousarray(proj_b.reshape(96, 1)).astype(np.float32)
    ridxs = [(128 * j + np.arange(128)).reshape(128, 1).astype(np.int32)
             for j in range(NCORES)]
    return xhs, wm, pw, pb, ridxs


def kernel(x, qkv_w, qkv_b, proj_w, proj_b, H=64, W=64):
    global _PROG
    if _PROG is None:
        _PROG = _build_program()
    nc = _PROG

    xhs, wm, pw, pb, ridxs = _prep_inputs(x, qkv_w, qkv_b, proj_w, proj_b)
    in_maps = [
        {"xh": xhs[i], "wm": wm, "pw": pw, "pb": pb, "ridx": ridxs[i]}
        for i in range(NCORES)
    ]
    res = run_bass_kernel_spmd(nc, in_maps, list(range(NCORES)))
    y = np.concatenate([np.asarray(res.results[i]["y"]) for i in range(NCORES)],
                       axis=1)
    return y


# revision 22
# speedup vs baseline: 1.3063x; 1.2792x over previous
"""ConvAttention TRN2 kernel: depthwise-conv QKV + full softmax attention + projection.

Self-contained: hardcodes shapes B=2, C=96, H=W=64, N=4096, heads=3, d=32.
v2 design:
  - Each core computes q/k/v conv ONLY for its own 512-token block (8 spatial
    rows + halo); k and v^T blocks are AllGathered (bf16) so every core holds
    full k / v for its 512 query rows.
  - Attention: per (b,h): 32 QK chunk matmuls -> exp (scalar engine) -> PV
    accumulation with augmented ones-row for the softmax denominator.
  - Normalization is issued deferred (reciprocal on vector, broadcast via a
    tiny matmul placed behind the next head's QK stream) so the in-order
    tensor queue never stalls on it.
  - Projection is local per core (per-head accumulating matmuls); the full
    output is assembled host-side from the 8 cores' y slices.
"""

import os
import sys

import numpy as np

for _p in ("/opt/trn_rl_repo", "/root/.axon_site/_ro/trn_rl_repo"):
    if os.path.isdir(_p) and _p not in sys.path:
        sys.path.append(_p)

from contextlib import ExitStack

import concourse.bass as bass
import concourse.masks as masks
import concourse.tile as tile
from concourse import bacc, mybir
from concourse.bass_utils import run_bass_kernel_spmd

F32 = mybir.dt.float32
BF16 = mybir.dt.bfloat16

B = 2
C = 96
H = W = 64
N = H * W            # 4096
NHEADS = 3
D = C // NHEADS      # 32
SCALE = float(D) ** -0.5
NCORES = 8
NQ = N // NCORES     # 512 query rows per core
QROWS = NQ // W      # 8 spatial rows per core
WP = W + 2           # padded width 66
NKCH = N // 128      # 32 key chunks of 128
HROWS = QROWS + 2    # halo rows per core
LH = QROWS * WP      # 528 usable elems per dy shift


def _build_program(debug_outputs=False):
    nc = bacc.Bacc("TRN2", target_bir_lowering=False, debug=False, num_devices=NCORES)

    xh_d = nc.dram_tensor("xh", [B, 96, HROWS, WP], BF16, kind="ExternalInput").ap()
    wm_d = nc.dram_tensor("wm", [3, 3, 97, 96], BF16, kind="ExternalInput").ap()
    pw_d = nc.dram_tensor("pw", [96, 96], BF16, kind="ExternalInput").ap()
    pb_d = nc.dram_tensor("pb", [96, 1], F32, kind="ExternalInput").ap()
    ridx_d = nc.dram_tensor("ridx", [128, 1], mybir.dt.int32,
                            kind="ExternalInput").ap()
    y_d = nc.dram_tensor("y", [B, NQ, 96], F32, kind="ExternalOutput").ap()

    kstg_d = [nc.dram_tensor(f"kstg{b}", [96, NQ], BF16).ap() for b in range(B)]
    vstg_d = [nc.dram_tensor(f"vstg{b}", [128, NHEADS, 4, 33], BF16).ap()
              for b in range(B)]
    astg_d = [[nc.dram_tensor(f"astg{b}_{h}", [32, NQ], BF16).ap()
               for h in range(NHEADS)] for b in range(B)]
    kg_d = [nc.dram_tensor(f"kg{b}", [NCORES, 96, NQ], BF16,
                           addr_space="Shared").ap() for b in range(B)]
    vg_d = [nc.dram_tensor(f"vg{b}", [NCORES, 128, NHEADS, 4, 33], BF16,
                           addr_space="Shared").ap() for b in range(B)]
    ag_d = [[nc.dram_tensor(f"ag{b}_{h}", [NCORES, 32, NQ], BF16,
                            addr_space="Shared").ap()
             for h in range(NHEADS)] for b in range(B)]
    flat_d = [nc.dram_tensor(f"flat{b}", [96, N], BF16).ap() for b in range(B)]
    dbg = {}
    if debug_outputs:
        dbg["q"] = nc.dram_tensor("dbg_q", [96, B, NQ], BF16, kind="ExternalOutput").ap()
        dbg["k"] = nc.dram_tensor("dbg_k", [96, B, N], BF16, kind="ExternalOutput").ap()
        dbg["vt"] = nc.dram_tensor("dbg_vt", [128, B, NHEADS, NKCH, 33], BF16, kind="ExternalOutput").ap()
        dbg["ah"] = nc.dram_tensor("dbg_ah", [B, NHEADS, 32, 512], F32, kind="ExternalOutput").ap()

    with tile.TileContext(nc) as tc, ExitStack() as ctx:
        consts = ctx.enter_context(tc.tile_pool(name="consts", bufs=1))
        xrep_p = ctx.enter_context(tc.tile_pool(name="xrep", bufs=1))
        qkv_p = ctx.enter_context(tc.tile_pool(name="qkv", bufs=1))
        vtmp_p = ctx.enter_context(tc.tile_pool(name="vtmp", bufs=2))
        exp_p = ctx.enter_context(tc.tile_pool(name="exp", bufs=18))
        small_p = ctx.enter_context(tc.tile_pool(name="small", bufs=3))
        ah_p = ctx.enter_context(tc.tile_pool(name="ah", bufs=6))
        out_p = ctx.enter_context(tc.tile_pool(name="out", bufs=2))

        acc_ps = ctx.enter_context(tc.tile_pool(name="acc_ps", bufs=2, space="PSUM"))
        sc_ps = ctx.enter_context(tc.tile_pool(name="sc_ps", bufs=2, space="PSUM"))
        misc_ps = ctx.enter_context(tc.tile_pool(name="misc_ps", bufs=2, space="PSUM"))

        # ---- constants ----
        wm_sb = consts.tile([97, 9, 96], BF16)
        for g in range(3):
            for dx in range(3):
                nc.sync.dma_start(wm_sb[:, g * 3 + dx, :], wm_d[g, dx, :, :])
        pw_sb = consts.tile([96, 96], BF16)
        nc.sync.dma_start(pw_sb[:, :], pw_d[:, :])
        pb_sb = consts.tile([96, 1], F32)
        nc.sync.dma_start(pb_sb[:], pb_d[:, :])
        ridx_sb = consts.tile([128, 1], mybir.dt.int32)
        nc.sync.dma_start(ridx_sb[:, :], ridx_d[:, :])
        ident = consts.tile([128, 128], BF16)
        masks.make_identity(nc, ident[:])
        ones_col = consts.tile([1, 32], F32)
        nc.vector.memset(ones_col[:], 1.0)

        # ---- halo input, replicated-shift layout: partition dy*32+c = channel c shifted dy rows ----
        xr = {}
        for gname, g in (("q", 0), ("k", 1), ("v", 2)):
            t = xrep_p.tile([97, B, LH], BF16, tag=f"x{gname}")
            xr[gname] = t
            flat = xh_d[:, g * 32:(g + 1) * 32, :, :].rearrange("b c r w -> c b (r w)")
            for dy in range(3):
                nc.sync.dma_start(t[dy * 32:(dy + 1) * 32, :, :],
                                  flat[:, :, dy * WP: dy * WP + LH])
            nc.vector.memset(t[96:97, :, :], 1.0)

        # ---- conv for one group/batch: psum [96, 512] ----
        def conv(g, xrt, b):
            view = xrt[:, b, :].rearrange("k (r w) -> k r w", w=WP)
            ps = acc_ps.tile([96, 512], F32, tag="pacc")
            for dx in range(3):
                nc.tensor.matmul(ps[:, :], lhsT=wm_sb[:, g * 3 + dx, :],
                                 rhs=view[:, 0:QROWS, dx: dx + W],
                                 start=(dx == 0), stop=(dx == 2))
            return ps

        # ---- per-b conv -> stage -> gather; order k0, q, v0, k1, v1 so the
        # collectives the attention needs first launch first ----
        kblk = qkv_p.tile([96, B, 512], BF16, tag="kblk")
        q_small = qkv_p.tile([96, B, 512], BF16, tag="qsm")
        vstg_sb = qkv_p.tile([128, B, NHEADS, 4, 33], BF16, tag="vstg")
        nc.vector.memset(vstg_sb[:, :, :, :, 32:33], 1.0)

        def emit_kconv(b):
            ps = conv(1, xr["k"], b)
            nc.vector.tensor_copy(kblk[:, b, :], ps[:, :])
            nc.sync.dma_start(kstg_d[b][:, :], kblk[:, b, :])
            nc.gpsimd.collective_compute(
                "AllGather", mybir.AluOpType.bypass,
                ins=[kstg_d[b][:, :]],
                outs=[kg_d[b][:, :, :]],
                replica_groups=[list(range(NCORES))])

        def emit_vconv(b):
            ps = conv(2, xr["v"], b)
            vtmp = vtmp_p.tile([96, 512], BF16)
            nc.scalar.copy(vtmp[:, :], ps[:, :])
            for c4 in range(4):
                tp = misc_ps.tile([128, 96], BF16, tag="m")
                nc.tensor.transpose(tp[:, :], vtmp[:, c4 * 128:(c4 + 1) * 128],
                                    ident[0:96, 0:96])
                nc.vector.tensor_copy(
                    vstg_sb[:, b, :, c4, 0:32],
                    tp[:, :].rearrange("p (h d) -> p h d", d=32))
            nc.sync.dma_start(vstg_d[b][:, :, :, :], vstg_sb[:, b, :, :, :])
            nc.gpsimd.collective_compute(
                "AllGather", mybir.AluOpType.bypass,
                ins=[vstg_d[b][:, :, :, :]],
                outs=[vg_d[b][:, :, :, :, :]],
                replica_groups=[list(range(NCORES))])

        emit_kconv(0)
        for b in range(B):
            ps = conv(0, xr["q"], b)
            nc.vector.tensor_copy(q_small[:, b, :], ps[:, :])
        emit_vconv(0)
        emit_kconv(1)
        emit_vconv(1)

        # ---- gather reads, all on sync in need-order (scalar stays pure exp) ----
        k_all = qkv_p.tile([96, B, N], BF16, tag="kall")
        vt_aug = qkv_p.tile([128, B, NHEADS, NKCH, 33], BF16, tag="vt")
        for b in range(B):
            nc.sync.dma_start(
                k_all[:, b, :].rearrange("c (j n) -> c j n", j=NCORES),
                kg_d[b][:, :, :].rearrange("j c n -> c j n"))
            nc.sync.dma_start(
                vt_aug[:, b, :, :, :].rearrange("p h (j c) s -> p h j c s", j=NCORES),
                vg_d[b][:, :, :, :, :].rearrange("j p h c s -> p h j c s"))

        # ---- attention ----
        BH = [(b, h) for b in range(B) for h in range(NHEADS)]
        pending = [None]   # (acc, rden, b, h)

        def flush_norm():
            if pending[0] is None:
                return
            acc, rden, b, h = pending[0]
            pending[0] = None
            bc = misc_ps.tile([32, 512], F32, tag="m")
            nc.tensor.matmul(bc[:, :], lhsT=ones_col[:, :], rhs=rden[:, :],
                             start=True, stop=True)
            bc_sb = small_p.tile([32, 512], F32, tag="bcsb")
            nc.vector.tensor_copy(bc_sb[:, :], bc[:, :])
            ah = ah_p.tile([32, 512], BF16, tag=f"a{b}_{h}")
            nc.vector.tensor_mul(ah[:, :], acc[0:32, :], bc_sb[:, :])
            nc.sync.dma_start(astg_d[b][h][:, :], ah[:, :])
            nc.gpsimd.collective_compute(
                "AllGather", mybir.AluOpType.bypass,
                ins=[astg_d[b][h][:, :]],
                outs=[ag_d[b][h][:, :, :]],
                replica_groups=[list(range(NCORES))])
            nc.gpsimd.dma_start(
                flat_d[b][32 * h:32 * (h + 1), :].rearrange(
                    "c (i n) -> c i n", i=NCORES),
                ag_d[b][h][:, :, :].rearrange("i c n -> c i n"))
            if debug_outputs:
                ahf = small_p.tile([32, 512], F32, tag="dbgah")
                nc.vector.tensor_copy(ahf[:, :], ah[:, :])
                nc.sync.dma_start(dbg["ah"][b, h, :, :], ahf[:, :])
            if h == NHEADS - 1:
                emit_proj(b)

        def emit_proj(b):
            # reference reshape(B, N, C) flattens (h, d, n) row-major: output
            # row n' = flat[96n' : 96n'+96]. This core projects rows
            # [512j, 512j+512): one indirect gather of 128 rows x 384 elems
            # (4 windows each) using the per-core row-index input.
            win = out_p.tile([128, 4, 96], BF16, tag="win")
            nc.gpsimd.indirect_dma_start(
                out=win[:, :, :].rearrange("p g c -> p (g c)"),
                out_offset=None,
                in_=flat_d[b][:, :].rearrange("c n -> (c n)").rearrange(
                    "(r e) -> r e", e=384),
                in_offset=bass.IndirectOffsetOnAxis(ap=ridx_sb[:, :1], axis=0))
            rhs = out_p.tile([96, 512], BF16, tag="prhs")
            for g in range(4):
                tpi = misc_ps.tile([96, 128], BF16, tag="m")
                nc.tensor.transpose(tpi[:, :], win[:, g, :], ident[:, :])
                nc.vector.tensor_copy(rhs[:, g * 128:(g + 1) * 128], tpi[:, :])
            y_ps = misc_ps.tile([96, 512], F32, tag="m")
            nc.tensor.matmul(y_ps[:, :], lhsT=pw_sb[:, :], rhs=rhs[:, :],
                             start=True, stop=True)
            ysb = out_p.tile([96, 512], BF16, tag="ysb")
            nc.vector.tensor_scalar_add(ysb[:, :], y_ps[:, :], pb_sb[:, :])
            yo = out_p.tile([128, 4, 96], F32, tag="yo")
            for g in range(4):
                tp = misc_ps.tile([128, 96], BF16, tag="m")
                nc.tensor.transpose(tp[:, :], ysb[:, g * 128:(g + 1) * 128],
                                    ident[0:96, 0:96])
                nc.vector.tensor_copy(yo[:, g, :], tp[:, :])
            nc.sync.dma_start(
                y_d[b].rearrange("(p g) c -> p g c", g=4), yo[:, :, :])

        for b, h in BH:
            # stage 1: QK chunk matmuls, paired into 2-bank psum tiles + exp
            exs = []
            for cp in range(NKCH // 2):
                sc2 = sc_ps.tile([128, 1024], F32)
                for half in range(2):
                    ch = 2 * cp + half
                    nc.tensor.matmul(
                        sc2[:, half * 512:(half + 1) * 512],
                        lhsT=k_all[32 * h:32 * (h + 1), b, ch * 128:(ch + 1) * 128],
                        rhs=q_small[32 * h:32 * (h + 1), b, :],
                        start=True, stop=True)
                ex2 = exp_p.tile([128, 1024], BF16)
                nc.scalar.activation(ex2[:, :], sc2[:, :],
                                     mybir.ActivationFunctionType.Exp, scale=SCALE)
                exs.append(ex2)
                if cp == 3:
                    # previous head's normalization: bc matmul lands behind 8
                    # queued QK matmuls, after its reciprocal has finished
                    flush_norm()
            # stage 2: PV accumulation (row 32 = sum of exp)
            acc = acc_ps.tile([33, 512], F32, tag="pacc")
            for cp in range(NKCH // 2):
                for half in range(2):
                    ch = 2 * cp + half
                    nc.tensor.matmul(
                        acc[:, :],
                        lhsT=vt_aug[:, b, h, ch, :],
                        rhs=exs[cp][:, half * 512:(half + 1) * 512],
                        start=(ch == 0), stop=(ch == NKCH - 1))
            den_sb = small_p.tile([1, 512], F32, tag="densb")
            nc.vector.tensor_copy(den_sb[:, :], acc[32:33, :])
            rden = small_p.tile([1, 512], F32, tag="rden")
            nc.vector.reciprocal_approx_fast(rden[:, :], den_sb[:, :])
            pending[0] = (acc, rden, b, h)
        flush_norm()

        if debug_outputs:
            nc.sync.dma_start(dbg["q"][:, :, :], q_small[:, :, :])
            nc.sync.dma_start(dbg["k"][:, :, :], k_all[:, :, :])
            nc.sync.dma_start(dbg["vt"][:, :, :, :, :], vt_aug[:, :, :, :, :])

    nc.compile()
    return nc


_PROG = None


def _prep_inputs(x, qkv_w, qkv_b, proj_w, proj_b):
    import ml_dtypes
    bf16 = ml_dtypes.bfloat16

    x = np.asarray(x, np.float32)
    qkv_w = np.asarray(qkv_w, np.float32)
    qkv_b = np.asarray(qkv_b, np.float32)
    proj_w = np.asarray(proj_w, np.float32)
    proj_b = np.asarray(proj_b, np.float32)

    xt = x.transpose(0, 2, 1).reshape(B, C, H, W)
    xpad = np.zeros((B, C, H + 2, WP), np.float32)
    xpad[:, :, 1:H + 1, 1:W + 1] = xt
    xpad = xpad.astype(bf16)

    xhs = [np.ascontiguousarray(xpad[:, :, i * QROWS: i * QROWS + HROWS, :])
           for i in range(NCORES)]

    w = qkv_w.reshape(3 * C, 3, 3)
    wm = np.zeros((3, 3, 97, 96), np.float32)  # [g, dx, k=(dy*32+c), o]
    o = np.arange(96)
    for g in range(3):
        for dy in range(3):
            for dx in range(3):
                wm[g, dx, dy * 32 + o // 3, o] = w[g * 96 + o, dy, dx]
        wm[g, 0, 96, :] = qkv_b[g * 96:(g + 1) * 96]
    wm = wm.astype(bf16)

    pw = np.ascontiguousarray(proj_w.T).astype(bf16)
    pb = np.ascontiguousarray(proj_b.reshape(96, 1)).astype(np.float32)
    ridxs = [(128 * j + np.arange(128)).reshape(128, 1).astype(np.int32)
             for j in range(NCORES)]
    return xhs, wm, pw, pb, ridxs


def kernel(x, qkv_w, qkv_b, proj_w, proj_b, H=64, W=64):
    global _PROG
    if _PROG is None:
        _PROG = _build_program()
    nc = _PROG

    xhs, wm, pw, pb, ridxs = _prep_inputs(x, qkv_w, qkv_b, proj_w, proj_b)
    in_maps = [
        {"xh": xhs[i], "wm": wm, "pw": pw, "pb": pb, "ridx": ridxs[i]}
        for i in range(NCORES)
    ]
    res = run_bass_kernel_spmd(nc, in_maps, list(range(NCORES)))
    y = np.concatenate([np.asarray(res.results[i]["y"]) for i in range(NCORES)],
                       axis=1)
    return y
